# revision 1
# baseline (speedup 1.0000x reference)
"""Trainium2 Bass kernel for nn_FeatLUT (embedding_lookup -> global mean).

Contract: kernel(**inputs) takes the FULL inputs from setup_inputs() and
returns the FULL (1, 20, 1, 1) float32 output; internally shards row-wise
across 8 NeuronCores (SPMD) and gathers/finishes on host.

Algorithm (per core, 256 rows x 2048 cols of both images):
  * The reference gathers feature[idx] with idx = 16*(289*x0 + 17*x1 + x2)
    (the channel weights 4624/272/16 share the factor 16), so only every
    16th LUT row is reachable: effective LUT16 = LUT[::16], 4913 rows.
  * Only the global mean is needed, so sum_p LUT16[j_p] = hist @ LUT16
    where hist is the 4913-bin index histogram. Per core we build hist as
    a 71x71 2-D histogram (q = j // 71, r = j % 71) with one-hot matmuls
    accumulated on the TensorEngine in PSUM (exact integer counts in f32).
  * One-hot construction is the bottleneck; this fabric's DVE runs at
    1 elem/part/cycle regardless of dtype/mode (measured), so per X=128
    columns ONE wide tensor_tensor(is_equal) compares a tiled constant
    C = [0..70, 0..70] against a zero-step replicated read of the
    interleaved bf16 (q, r) tile -> [128, X, 142] one-hot pairs at
    ~142 DVE cycles per 128-pixel column (vs ~283 for per-column
    tensor_scalar pairs).
  * q is computed exactly in f32: round_to_nearest(j/71 - 0.4965) via the
    +-1.5*2^23 magic-add trick (fractions k/71 keep >=0.0035 margin from
    rounding boundaries, >> fp32 error).
  * hist is contracted with the rearranged LUT16 on-chip (142 small f32
    matmuls into a [1,20] PSUM accumulator); host sums the 8 per-core
    partials and applies mean -> *4 -> round -> /4 -> clamp.
"""

import sys

sys.path.insert(0, "/opt/trn_rl_repo")

import numpy as np

N_CORES = 8
H = W = 2048
ROWS = H // N_CORES  # 256
CC = 2048  # column chunk (full row width)
XW = 128  # columns per wide one-hot op (fewer, bigger DVE ops: ~2.5us hidden per-op cost)
QB = 71
RB = 71
W2 = 2 * QB
NFEAT = 20
MAGIC = 12582912.0  # 1.5 * 2^23

LAST_EXEC_NS = None
LAST_TRACE = None
TRACE = False
_CACHED = None


def _build():
    from contextlib import ExitStack

    import concourse.bacc as bacc
    import concourse.bass as bass
    import concourse.mybir as mybir
    import concourse.tile as tile

    f32 = mybir.dt.float32
    bf16 = mybir.dt.bfloat16
    A = mybir.AluOpType

    nc = bacc.Bacc("TRN2", target_bir_lowering=False, debug=False)
    xin = nc.dram_tensor("xin", [3, ROWS, W], f32, kind="ExternalInput")
    xs = nc.dram_tensor("xs", [3, ROWS, W], f32, kind="ExternalInput")
    tmsb = nc.dram_tensor("tmsb", [QB, RB * NFEAT], f32, kind="ExternalInput")
    tlsb = nc.dram_tensor("tlsb", [QB, RB * NFEAT], f32, kind="ExternalInput")
    out = nc.dram_tensor("out", [1, NFEAT], f32, kind="ExternalOutput")

    n_rb = ROWS // 128
    n_cc = W // CC

    with tile.TileContext(nc) as tc:
        with ExitStack() as ctx:
            singles = ctx.enter_context(tc.tile_pool(name="singles", bufs=1))
            xpool = ctx.enter_context(tc.tile_pool(name="xpool", bufs=2))
            prep = ctx.enter_context(tc.tile_pool(name="prep", bufs=2))
            ohp = ctx.enter_context(tc.tile_pool(name="ohp", bufs=2))
            psum = ctx.enter_context(tc.tile_pool(name="psum", bufs=1, space="PSUM"))

            # C[x, s, i] = i for s in {0,1}: [0..70, 0..70] per column slot
            C = singles.tile([128, W2], bf16)
            nc.gpsimd.iota(
                C,
                pattern=[[0, 2], [1, QB]],
                base=0,
                channel_multiplier=0,
                allow_small_or_imprecise_dtypes=True,
            )
            lut_m = singles.tile([QB, RB * NFEAT], f32)
            nc.sync.dma_start(out=lut_m, in_=tmsb[:, :])
            lut_l = singles.tile([QB, RB * NFEAT], f32)
            nc.sync.dma_start(out=lut_l, in_=tlsb[:, :])

            hist_m = psum.tile([QB, RB], f32)
            hist_l = psum.tile([QB, RB], f32)
            acc = psum.tile([1, NFEAT], f32)

            for xdram, hist in ((xin, hist_m), (xs, hist_l)):
                mm = 0
                total_mm = n_rb * n_cc * CC
                for rb in range(n_rb):
                    for ci in range(n_cc):
                        rs = slice(rb * 128, (rb + 1) * 128)
                        cs = slice(ci * CC, (ci + 1) * CC)
                        x0 = xpool.tile([128, CC], f32, tag="x0")
                        x1 = xpool.tile([128, CC], f32, tag="x1")
                        x2 = xpool.tile([128, CC], f32, tag="x2")
                        nc.sync.dma_start(out=x0, in_=xdram[0, rs, cs])
                        nc.sync.dma_start(out=x1, in_=xdram[1, rs, cs])
                        nc.sync.dma_start(out=x2, in_=xdram[2, rs, cs])

                        u = prep.tile([128, CC], f32, tag="u")
                        nc.vector.scalar_tensor_tensor(
                            out=u, in0=x0, scalar=17.0, in1=x1, op0=A.mult, op1=A.add
                        )
                        j = prep.tile([128, CC], f32, tag="j")
                        nc.vector.scalar_tensor_tensor(
                            out=j, in0=u, scalar=17.0, in1=x2, op0=A.mult, op1=A.add
                        )
                        t = prep.tile([128, CC], f32, tag="u")
                        nc.vector.tensor_scalar(
                            out=t,
                            in0=j,
                            scalar1=1.0 / 71.0,
                            scalar2=0.4965,
                            op0=A.mult,
                            op1=A.subtract,
                        )
                        qr = prep.tile([128, CC, 2], bf16, tag="qr")
                        qcol = bass.AP(
                            tensor=qr.tensor, offset=qr.offset, ap=[qr.ap[0], [2, CC]]
                        )
                        nc.vector.tensor_scalar(
                            out=qcol,
                            in0=t,
                            scalar1=MAGIC,
                            scalar2=MAGIC,
                            op0=A.add,
                            op1=A.subtract,
                        )
                        rcol = bass.AP(
                            tensor=qr.tensor,
                            offset=qr.offset + 1,
                            ap=[qr.ap[0], [2, CC]],
                        )
                        nc.vector.scalar_tensor_tensor(
                            out=rcol,
                            in0=qcol,
                            scalar=-float(QB),
                            in1=j,
                            op0=A.mult,
                            op1=A.add,
                        )

                        for g in range(CC // XW):
                            oh = ohp.tile([128, XW, W2], bf16, tag="oh")
                            c_view = bass.AP(
                                tensor=C.tensor,
                                offset=C.offset,
                                ap=[C.ap[0], [0, XW], [QB, 2], [1, QB]],
                            )
                            qr_view = bass.AP(
                                tensor=qr.tensor,
                                offset=qr.offset + g * 2 * XW,
                                ap=[qr.ap[0], [2, XW], [1, 2], [0, QB]],
                            )
                            oh_view = bass.AP(
                                tensor=oh.tensor,
                                offset=oh.offset,
                                ap=[oh.ap[0], [W2, XW], [QB, 2], [1, QB]],
                            )
                            nc.vector.tensor_tensor(
                                out=oh_view, in0=c_view, in1=qr_view, op=A.is_equal
                            )
                            for x in range(XW):
                                nc.tensor.matmul(
                                    hist[:, :],
                                    oh[:, x, 0:QB],
                                    oh[:, x, QB:W2],
                                    start=(mm == 0),
                                    stop=(mm == total_mm - 1),
                                )
                                mm += 1

            hist_m_sb = singles.tile([QB, RB], f32)
            nc.vector.tensor_copy(hist_m_sb, hist_m)
            hist_l_sb = singles.tile([QB, RB], f32)
            nc.vector.tensor_copy(hist_l_sb, hist_l)

            fm = 0
            for hist_sb, lut in ((hist_m_sb, lut_m), (hist_l_sb, lut_l)):
                for rr in range(RB):
                    nc.tensor.matmul(
                        acc[:, :],
                        hist_sb[:, rr : rr + 1],
                        lut[:, rr * NFEAT : (rr + 1) * NFEAT],
                        start=(fm == 0),
                        stop=(fm == 2 * RB - 1),
                    )
                    fm += 1

            out_sb = singles.tile([1, NFEAT], f32)
            nc.vector.tensor_copy(out_sb, acc)
            nc.sync.dma_start(out=out[:, :], in_=out_sb)

    nc.compile()
    return nc


def _prep_table(feat):
    """[78608,20,1,1] int8 -> [71, 71*20] f32 (LUT16 in q-major layout)."""
    t = np.asarray(feat).reshape(78608, NFEAT)[::16].astype(np.float32)
    pad = np.zeros((QB * RB, NFEAT), np.float32)
    pad[: t.shape[0]] = t
    return np.ascontiguousarray(pad.reshape(QB, RB * NFEAT))


def kernel(x_in, x_s, feature_msb, feature_lsb):
    global LAST_EXEC_NS, LAST_TRACE, _CACHED
    from concourse import bass_utils

    if _CACHED is None:
        _CACHED = _build()
    nc = _CACHED

    x_in = np.ascontiguousarray(np.asarray(x_in, dtype=np.float32).reshape(3, H, W))
    x_s = np.ascontiguousarray(np.asarray(x_s, dtype=np.float32).reshape(3, H, W))
    tm = _prep_table(feature_msb)
    tl = _prep_table(feature_lsb)

    in_maps = []
    for c in range(N_CORES):
        rs = slice(c * ROWS, (c + 1) * ROWS)
        in_maps.append(
            {
                "xin": np.ascontiguousarray(x_in[:, rs, :]),
                "xs": np.ascontiguousarray(x_s[:, rs, :]),
                "tmsb": tm,
                "tlsb": tl,
            }
        )

    try:
        res = bass_utils.run_bass_kernel_spmd(
            nc, in_maps, core_ids=list(range(N_CORES)), trace=TRACE
        )
    except Exception:
        # transient device errors (e.g. NRT_EXEC_UNIT_UNRECOVERABLE) have
        # been observed on this fabric; one retry clears them
        res = bass_utils.run_bass_kernel_spmd(
            nc, in_maps, core_ids=list(range(N_CORES)), trace=TRACE
        )
    LAST_EXEC_NS = res.exec_time_ns
    LAST_TRACE = res.instructions_and_trace

    s = np.zeros(NFEAT, np.float64)
    for rr in res.results:
        s += rr["out"].astype(np.float64).reshape(NFEAT)
    mean = s / float(H * W)
    q = np.clip(np.round(mean * 4.0) / 4.0, -32.0, 31.75)
    return q.reshape(1, NFEAT, 1, 1).astype(np.float32)

